# revision 2
# baseline (speedup 1.0000x reference)
"""Trainium2 Bass kernel: segment-mean -> gated MLP -> per-node modulation.

Computes, for h_V [N, D] and sorted batch_id [N] (values in [0, S)):
    seg_sum[s] = sum of h_V rows with batch_id == s ; counts[s]
    c_V = seg_sum / max(counts, 1)
    g   = sigmoid(relu(c_V @ W1 + b1) @ W2 + b2)
    out = h_V * g[batch_id]

Distribution: batch_id is SORTED, so rows of each segment are contiguous.
We shard by WHOLE segments: core c owns segments [8c, 8c+8) -- every
segment's mean is core-local, so there are NO collectives at all.

Per-core layout (host-marshalled, pure layout/dtype transform): each
segment t gets a fixed region of 128 rows x 128 partitions (16384-row
capacity, zero-padded).  Every SBUF partition therefore holds rows of
exactly ONE segment per region, which collapses the segment reduction to
a stream of wide accumulating matmuls with a constant [P,1] weight
column (64/R_t, so PSUM accumulates 64*mean directly; the 1/64 is folded
into W1 on the host).  The gate gather likewise collapses to a rank-1
broadcast matmul per segment, and the modulation pass reuses the SBUF-
resident fp16 tiles from pass 1 (zero re-read of h_V).

Only h_V's fp16 rounding (~5e-4 relative) is lossy; the harness
tolerance is 2e-2.
"""

import math

import numpy as np

# Problem constants (hardcoded per the harness contract).
D = 128  # feature dim
S = 64  # number of segments
P = 128  # SBUF partitions
N_CORES = 8
N_FULL = 1_000_000
SEGS_PER_CORE = S // N_CORES  # 8
ROWS_PP = 128  # rows per partition per segment region
CAP_SEG = P * ROWS_PP  # 16384-row capacity per segment
Q = SEGS_PER_CORE * ROWS_PP  # 1024 rows per partition per core
T_ROWS = 32  # rows per partition per macro DMA tile
MACROS_PER_SEG = ROWS_PP // T_ROWS  # 4
SEG_ELS = ROWS_PP * D  # 16384 fp16 els per partition per segment
MAC_ELS = T_ROWS * D  # 4096
CHUNK = 512  # fp16 els per matmul rhs (one f32 PSUM bank of output)
CH_PER_MAC = MAC_ELS // CHUNK  # 8


def segment_kernel(tc, outs, ins):
    """Emit the per-core Tile program (no cross-core communication)."""
    import concourse.mybir as mybir
    from concourse.bass import broadcast_tensor_aps

    nc = tc.nc
    F32 = mybir.dt.float32
    F16 = mybir.dt.float16
    AF = mybir.ActivationFunctionType
    OP = mybir.AluOpType

    hv = ins["hv16"]  # [P, Q*D] f16; per-partition: seg t, row i, d
    abar = ins["abar"]  # [P, SEGS_PER_CORE] f16: col t = 64/R_t
    w1q = ins["W1q"]  # [D, D] f32 = W1 / 64
    w2 = ins["W2"]  # [D, D] f32
    b1 = ins["b1"]  # [D] f32
    b2 = ins["b2"]  # [D] f32
    ident16 = ins["ident16"]  # [P, P] f16 identity
    ones11 = ins["ones11"]  # [1, 1] f32
    ones_row = ins["ones_row"]  # [1, P] f16
    out = outs["out"]  # [P, Q*D] f32

    with tc.tile_pool(name="pers", bufs=1) as pers:
        abar_sb = pers.tile_from(abar, name="abar_sb", force_copy=True)
        w1_sb = pers.tile_from(w1q, name="w1_sb", force_copy=True)
        w2_sb = pers.tile_from(w2, name="w2_sb", force_copy=True)
        ident_sb = pers.tile_from(ident16, name="ident_sb", force_copy=True)
        ones11_sb = pers.tile_from(ones11, name="ones11_sb", force_copy=True)
        onesrow_sb = pers.tile_from(ones_row, name="onesrow_sb", force_copy=True)
        b1_sb = pers.tile([P, 1], F32, name="b1_sb")
        nc.sync.dma_start(out=b1_sb, in_=b1)
        b2_sb = pers.tile([P, 1], F32, name="b2_sb")
        nc.sync.dma_start(out=b2_sb, in_=b2)

        with (
            tc.tile_pool(name="hvp", bufs=2 * MACROS_PER_SEG) as hvp,
            tc.tile_pool(name="outp", bufs=4) as outp,
            tc.tile_pool(name="gatep", bufs=2) as gatep,
            tc.tile_pool(name="mlpsb", bufs=2) as mlpsb,
            tc.tile_pool(name="accps", bufs=2, space="PSUM") as accps,
            tc.tile_pool(name="mlpps", bufs=2, space="PSUM") as mlpps,
            tc.tile_pool(name="gateps", bufs=2, space="PSUM") as gateps,
        ):
            for t in range(SEGS_PER_CORE):
                base = t * SEG_ELS
                # ---- pass 1: accumulate 64*mean_t into one PSUM bank ----
                acc = accps.tile([1, CHUNK], F32, tag="acc", name=f"acc{t}")
                hv_tiles = []
                n_ch = MACROS_PER_SEG * CH_PER_MAC
                ci = 0
                for m in range(MACROS_PER_SEG):
                    hv_t = hvp.tile([P, MAC_ELS], F16, tag="hv", name=f"hv{t}_{m}")
                    lo = base + m * MAC_ELS
                    nc.sync.dma_start(out=hv_t, in_=hv[:, lo : lo + MAC_ELS])
                    hv_tiles.append(hv_t)
                    for ch in range(CH_PER_MAC):
                        nc.tensor.matmul(
                            acc,
                            lhsT=abar_sb[:, t : t + 1],
                            rhs=hv_t[:, ch * CHUNK : (ch + 1) * CHUNK],
                            start=(ci == 0),
                            stop=(ci == n_ch - 1),
                            skip_group_check=True,
                        )
                        ci += 1

                # ---- fold the CHUNK//D phases: cv_row = 64*mean_t [1, D] ----
                cv_row = mlpsb.tile([1, D], F32, tag="cv", name=f"cv{t}")
                acc_v = acc.rearrange("p (g d) -> p d g", d=D)
                nc.vector.reduce_sum(
                    out=cv_row, in_=acc_v, axis=mybir.AxisListType.X
                )

                # ---- tiny per-segment MLP ----
                cvt_ps = mlpps.tile([D, 1], F32, tag="mlp", name=f"cvt_ps{t}")
                nc.tensor.matmul(cvt_ps, lhsT=cv_row, rhs=ones11_sb)
                cvt_sb = mlpsb.tile([D, 1], F32, tag="cvt", name=f"cvt{t}")
                nc.scalar.copy(cvt_sb, cvt_ps)
                h1_ps = mlpps.tile([D, 1], F32, tag="mlp", name=f"h1_ps{t}")
                nc.tensor.matmul(h1_ps, lhsT=w1_sb, rhs=cvt_sb)
                h1_sb = mlpsb.tile([D, 1], F32, tag="h1", name=f"h1{t}")
                nc.scalar.activation(h1_sb, h1_ps, AF.Relu, bias=b1_sb, scale=1.0)
                h2_ps = mlpps.tile([D, 1], F32, tag="mlp", name=f"h2_ps{t}")
                nc.tensor.matmul(h2_ps, lhsT=w2_sb, rhs=h1_sb)
                g_col = mlpsb.tile([D, 1], F16, tag="gc", name=f"gcol{t}")
                nc.scalar.activation(g_col, h2_ps, AF.Sigmoid, bias=b2_sb, scale=1.0)
                # g as a row: [1, D] = g_col^T via identity
                grow_ps = mlpps.tile([1, D], F32, tag="mlp", name=f"grow_ps{t}")
                nc.tensor.matmul(grow_ps, lhsT=g_col, rhs=ident_sb)
                g_row = mlpsb.tile([1, D], F16, tag="gr", name=f"grow{t}")
                nc.scalar.copy(g_row, grow_ps)
                # broadcast to all partitions: gate[p, d] = g[d]
                gate_ps = gateps.tile([P, D], F32, tag="gps", name=f"gate_ps{t}")
                nc.tensor.matmul(gate_ps, lhsT=onesrow_sb, rhs=g_row)
                gate_sb = gatep.tile([P, D], F32, tag="gate", name=f"gate{t}")
                nc.scalar.copy(gate_sb, gate_ps)

                # ---- pass 2: modulate the retained fp16 tiles, store ----
                gate3 = gate_sb.rearrange("p (o d) -> p o d", o=1)
                for m in range(MACROS_PER_SEG):
                    out_t = outp.tile([P, MAC_ELS], F32, tag="out", name=f"o{t}_{m}")
                    hv3 = hv_tiles[m].rearrange("p (r d) -> p r d", d=D)
                    in1, in2 = broadcast_tensor_aps(hv3, gate3)
                    nc.vector.tensor_tensor(
                        out_t.rearrange("p (r d) -> p r d", d=D), in1, in2, OP.mult
                    )
                    lo = base + m * MAC_ELS
                    nc.sync.dma_start(out=out[:, lo : lo + MAC_ELS], in_=out_t)


def build_nc():
    """Build the Bass module with ExternalInput/Output DRAM tensors."""
    import concourse.bacc as bacc
    import concourse.mybir as mybir
    import concourse.tile as tile

    F32 = mybir.dt.float32
    F16 = mybir.dt.float16
    nc = bacc.Bacc(
        "TRN2",
        target_bir_lowering=False,
        debug=False,
        enable_asserts=False,
        num_devices=N_CORES,
    )

    def din(name, shape, dt):
        return nc.dram_tensor(name, shape, dt, kind="ExternalInput").ap()

    ins = {
        "hv16": din("hv16", [P, Q * D], F16),
        "abar": din("abar", [P, SEGS_PER_CORE], F16),
        "W1q": din("W1q", [D, D], F32),
        "W2": din("W2", [D, D], F32),
        "b1": din("b1", [D], F32),
        "b2": din("b2", [D], F32),
        "ident16": din("ident16", [P, P], F16),
        "ones11": din("ones11", [1, 1], F32),
        "ones_row": din("ones_row", [1, P], F16),
    }
    outs = {"out": nc.dram_tensor("out", [P, Q * D], F32, kind="ExternalOutput").ap()}
    with tile.TileContext(nc) as tc:
        segment_kernel(tc, outs, ins)
    nc.compile()
    return nc


_NC_CACHE = {}


def _get_nc():
    if "nc" not in _NC_CACHE:
        _NC_CACHE["nc"] = build_nc()
    return _NC_CACHE["nc"]


def run(inputs, trace=False, trace_kwargs=None):
    from concourse import bass_utils

    h_V = np.asarray(inputs["h_V"], dtype=np.float32)
    bid = np.asarray(inputs["batch_id"]).astype(np.int64)
    n = h_V.shape[0]
    counts = np.bincount(bid, minlength=S)
    assert counts.max() <= CAP_SEG, f"segment too large: {counts.max()}"
    bounds = np.concatenate([[0], np.cumsum(counts)])
    h16 = h_V.astype(np.float16)

    weights = {
        "W1q": np.ascontiguousarray(np.asarray(inputs["W1"], np.float32)) / 64.0,
        "W2": np.ascontiguousarray(np.asarray(inputs["W2"], np.float32)),
        "b1": np.ascontiguousarray(np.asarray(inputs["b1"], np.float32)),
        "b2": np.ascontiguousarray(np.asarray(inputs["b2"], np.float32)),
        "ident16": np.eye(P, dtype=np.float16),
        "ones11": np.ones((1, 1), np.float32),
        "ones_row": np.ones((1, P), np.float16),
    }

    in_maps = []
    for c in range(N_CORES):
        hvc = np.zeros((SEGS_PER_CORE, CAP_SEG, D), np.float16)
        ab = np.zeros((P, SEGS_PER_CORE), np.float16)
        for t in range(SEGS_PER_CORE):
            s = c * SEGS_PER_CORE + t
            lo, hi = bounds[s], bounds[s + 1]
            hvc[t, : hi - lo] = h16[lo:hi]
            ab[:, t] = 64.0 / max(hi - lo, 1)
        hv_core = np.ascontiguousarray(
            hvc.reshape(SEGS_PER_CORE, P, ROWS_PP, D).transpose(1, 0, 2, 3)
        ).reshape(P, Q * D)
        in_maps.append({"hv16": hv_core, "abar": ab, **weights})

    nc = _get_nc()
    res = bass_utils.run_bass_kernel_spmd(
        nc,
        in_maps,
        core_ids=list(range(N_CORES)),
        trace=trace,
        **(trace_kwargs or {}),
    )

    out_full = np.empty((n, D), np.float32)
    for c in range(N_CORES):
        o = (
            np.asarray(res.results[c]["out"])
            .reshape(P, SEGS_PER_CORE, ROWS_PP, D)
            .transpose(1, 0, 2, 3)
            .reshape(SEGS_PER_CORE, CAP_SEG, D)
        )
        for t in range(SEGS_PER_CORE):
            s = c * SEGS_PER_CORE + t
            lo, hi = bounds[s], bounds[s + 1]
            out_full[lo:hi] = o[t, : hi - lo]
    return out_full, res


def kernel(**inputs) -> np.ndarray:
    out, _ = run(inputs, trace=False)
    return out


# revision 7
# speedup vs baseline: 1.0741x; 1.0741x over previous
"""Trainium2 Bass kernel: segment-mean -> gated MLP -> per-node modulation.

Computes, for h_V [N, D] and sorted batch_id [N] (values in [0, S)):
    seg_sum[s] = sum of h_V rows with batch_id == s ; counts[s]
    c_V = seg_sum / max(counts, 1)
    g   = sigmoid(relu(c_V @ W1 + b1) @ W2 + b2)
    out = h_V * g[batch_id]

Distribution: batch_id is SORTED, so rows of each segment are contiguous.
We shard by WHOLE segments: core c owns segments [8c, 8c+8) -- every
segment's mean is core-local, so there are NO collectives at all.

Per-core layout (host-marshalled, pure layout/dtype transform): each
segment t gets a fixed region of 128 rows x 128 partitions (16384-row
capacity, zero-padded).  Every SBUF partition therefore holds rows of
exactly ONE segment per region, which collapses the segment reduction to
a stream of wide accumulating matmuls with a constant [P,1] weight
column (64/R_t, so PSUM accumulates 64*mean directly; the 1/64 is folded
into W1 on the host).  The gate gather likewise collapses to a rank-1
broadcast matmul per segment, and the modulation pass reuses the SBUF-
resident fp16 tiles from pass 1 (zero re-read of h_V).

Only h_V's fp16 rounding (~5e-4 relative) is lossy; the harness
tolerance is 2e-2.
"""

import math

import numpy as np

# Problem constants (hardcoded per the harness contract).
D = 128  # feature dim
S = 64  # number of segments
P = 128  # SBUF partitions
N_CORES = 8
N_FULL = 1_000_000
SEGS_PER_CORE = S // N_CORES  # 8
ROWS_PP = 128  # rows per partition per segment region
CAP_SEG = P * ROWS_PP  # 16384-row capacity per segment
Q = SEGS_PER_CORE * ROWS_PP  # 1024 rows per partition per core
T_ROWS = 32  # rows per partition per macro DMA tile
MACROS_PER_SEG = ROWS_PP // T_ROWS  # 4
SEG_ELS = ROWS_PP * D  # 16384 fp16 els per partition per segment
MAC_ELS = T_ROWS * D  # 4096
CHUNK = 512  # fp16 els per matmul rhs (one f32 PSUM bank of output)
CH_PER_MAC = MAC_ELS // CHUNK  # 8


def segment_kernel(tc, outs, ins):
    """Emit the per-core Tile program (no cross-core communication)."""
    import concourse.mybir as mybir
    from concourse.bass import broadcast_tensor_aps

    nc = tc.nc
    F32 = mybir.dt.float32
    F16 = mybir.dt.float16
    AF = mybir.ActivationFunctionType
    OP = mybir.AluOpType

    hv = ins["hv16"]  # [P, Q*D] f16; per-partition: seg t, row i, d
    abar = ins["abar"]  # [P, SEGS_PER_CORE] f16: col t = 64/R_t
    w1q = ins["W1q"]  # [D, D] f32 = W1 / 64
    w2 = ins["W2"]  # [D, D] f32
    b1 = ins["b1"]  # [D] f32
    b2 = ins["b2"]  # [D] f32
    ident16 = ins["ident16"]  # [P, P] f16 identity
    ones11 = ins["ones11"]  # [1, 1] f32
    ones_row = ins["ones_row"]  # [1, P] f16
    out = outs["out"]  # [P, Q*D] f16 (host upcasts to f32)

    with tc.tile_pool(name="pers", bufs=1) as pers:
        abar_sb = pers.tile_from(abar, name="abar_sb", force_copy=True)
        w1_sb = pers.tile_from(w1q, name="w1_sb", force_copy=True)
        w2_sb = pers.tile_from(w2, name="w2_sb", force_copy=True)
        ident_sb = pers.tile_from(ident16, name="ident_sb", force_copy=True)
        ones11_sb = pers.tile_from(ones11, name="ones11_sb", force_copy=True)
        onesrow_sb = pers.tile_from(ones_row, name="onesrow_sb", force_copy=True)
        b1_sb = pers.tile([P, 1], F32, name="b1_sb")
        nc.sync.dma_start(out=b1_sb, in_=b1)
        b2_sb = pers.tile([P, 1], F32, name="b2_sb")
        nc.sync.dma_start(out=b2_sb, in_=b2)

        with (
            tc.tile_pool(name="hvp", bufs=2 * MACROS_PER_SEG) as hvp,
            tc.tile_pool(name="outp", bufs=4) as outp,
            tc.tile_pool(name="gatep", bufs=2) as gatep,
            tc.tile_pool(name="mlpsb", bufs=2) as mlpsb,
            tc.tile_pool(name="accps", bufs=2, space="PSUM") as accps,
            tc.tile_pool(name="mlpps", bufs=2, space="PSUM") as mlpps,
            tc.tile_pool(name="gateps", bufs=2, space="PSUM") as gateps,
        ):
            for t in range(SEGS_PER_CORE):
                base = t * SEG_ELS
                # ---- pass 1: accumulate 64*mean_t into one PSUM bank ----
                acc = accps.tile([1, CHUNK], F32, tag="acc", name=f"acc{t}")
                hv_tiles = []
                n_ch = MACROS_PER_SEG * CH_PER_MAC
                ci = 0
                for m in range(MACROS_PER_SEG):
                    hv_t = hvp.tile([P, MAC_ELS], F16, tag="hv", name=f"hv{t}_{m}")
                    lo = base + m * MAC_ELS
                    nc.sync.dma_start(out=hv_t, in_=hv[:, lo : lo + MAC_ELS])
                    hv_tiles.append(hv_t)
                    for ch in range(CH_PER_MAC):
                        nc.tensor.matmul(
                            acc,
                            lhsT=abar_sb[:, t : t + 1],
                            rhs=hv_t[:, ch * CHUNK : (ch + 1) * CHUNK],
                            start=(ci == 0),
                            stop=(ci == n_ch - 1),
                            skip_group_check=True,
                        )
                        ci += 1

                # ---- fold the CHUNK//D phases: cv_row = 64*mean_t [1, D] ----
                cv_row = mlpsb.tile([1, D], F32, tag="cv", name=f"cv{t}")
                acc_v = acc.rearrange("p (g d) -> p d g", d=D)
                nc.vector.reduce_sum(
                    out=cv_row, in_=acc_v, axis=mybir.AxisListType.X
                )

                # ---- tiny per-segment MLP ----
                cvt_ps = mlpps.tile([D, 1], F32, tag="mlp", name=f"cvt_ps{t}")
                nc.tensor.matmul(cvt_ps, lhsT=cv_row, rhs=ones11_sb)
                cvt_sb = mlpsb.tile([D, 1], F32, tag="cvt", name=f"cvt{t}")
                nc.scalar.copy(cvt_sb, cvt_ps)
                h1_ps = mlpps.tile([D, 1], F32, tag="mlp", name=f"h1_ps{t}")
                nc.tensor.matmul(h1_ps, lhsT=w1_sb, rhs=cvt_sb)
                h1_sb = mlpsb.tile([D, 1], F32, tag="h1", name=f"h1{t}")
                nc.scalar.activation(h1_sb, h1_ps, AF.Relu, bias=b1_sb, scale=1.0)
                h2_ps = mlpps.tile([D, 1], F32, tag="mlp", name=f"h2_ps{t}")
                nc.tensor.matmul(h2_ps, lhsT=w2_sb, rhs=h1_sb)
                g_col = mlpsb.tile([D, 1], F16, tag="gc", name=f"gcol{t}")
                nc.scalar.activation(g_col, h2_ps, AF.Sigmoid, bias=b2_sb, scale=1.0)
                # g as a row: [1, D] = g_col^T via identity
                grow_ps = mlpps.tile([1, D], F32, tag="mlp", name=f"grow_ps{t}")
                nc.tensor.matmul(grow_ps, lhsT=g_col, rhs=ident_sb)
                g_row = mlpsb.tile([1, D], F16, tag="gr", name=f"grow{t}")
                nc.scalar.copy(g_row, grow_ps)
                # broadcast to all partitions: gate[p, d] = g[d]
                gate_ps = gateps.tile([P, D], F32, tag="gps", name=f"gate_ps{t}")
                nc.tensor.matmul(gate_ps, lhsT=onesrow_sb, rhs=g_row)
                gate_sb = gatep.tile([P, D], F16, tag="gate", name=f"gate{t}")
                nc.scalar.copy(gate_sb, gate_ps)

                # ---- pass 2: modulate the retained fp16 tiles, store ----
                gate3 = gate_sb.rearrange("p (o d) -> p o d", o=1)
                for m in range(MACROS_PER_SEG):
                    out_t = outp.tile([P, MAC_ELS], F16, tag="out", name=f"o{t}_{m}")
                    hv3 = hv_tiles[m].rearrange("p (r d) -> p r d", d=D)
                    in1, in2 = broadcast_tensor_aps(hv3, gate3)
                    nc.vector.tensor_tensor(
                        out_t.rearrange("p (r d) -> p r d", d=D), in1, in2, OP.mult
                    )
                    lo = base + m * MAC_ELS
                    nc.sync.dma_start(out=out[:, lo : lo + MAC_ELS], in_=out_t)


def build_nc():
    """Build the Bass module with ExternalInput/Output DRAM tensors."""
    import concourse.bacc as bacc
    import concourse.mybir as mybir
    import concourse.tile as tile

    F32 = mybir.dt.float32
    F16 = mybir.dt.float16
    nc = bacc.Bacc(
        "TRN2",
        target_bir_lowering=False,
        debug=False,
        enable_asserts=False,
        num_devices=N_CORES,
    )

    def din(name, shape, dt):
        return nc.dram_tensor(name, shape, dt, kind="ExternalInput").ap()

    ins = {
        "hv16": din("hv16", [P, Q * D], F16),
        "abar": din("abar", [P, SEGS_PER_CORE], F16),
        "W1q": din("W1q", [D, D], F32),
        "W2": din("W2", [D, D], F32),
        "b1": din("b1", [D], F32),
        "b2": din("b2", [D], F32),
        "ident16": din("ident16", [P, P], F16),
        "ones11": din("ones11", [1, 1], F32),
        "ones_row": din("ones_row", [1, P], F16),
    }
    outs = {"out": nc.dram_tensor("out", [P, Q * D], F16, kind="ExternalOutput").ap()}
    with tile.TileContext(nc) as tc:
        segment_kernel(tc, outs, ins)
    nc.compile()
    return nc


_NC_CACHE = {}


def _get_nc():
    if "nc" not in _NC_CACHE:
        _NC_CACHE["nc"] = build_nc()
    return _NC_CACHE["nc"]


def run(inputs, trace=False, trace_kwargs=None):
    from concourse import bass_utils

    h_V = np.asarray(inputs["h_V"], dtype=np.float32)
    bid = np.asarray(inputs["batch_id"]).astype(np.int64)
    n = h_V.shape[0]
    counts = np.bincount(bid, minlength=S)
    assert counts.max() <= CAP_SEG, f"segment too large: {counts.max()}"
    bounds = np.concatenate([[0], np.cumsum(counts)])
    h16 = h_V.astype(np.float16)

    weights = {
        "W1q": np.ascontiguousarray(np.asarray(inputs["W1"], np.float32)) / 64.0,
        "W2": np.ascontiguousarray(np.asarray(inputs["W2"], np.float32)),
        "b1": np.ascontiguousarray(np.asarray(inputs["b1"], np.float32)),
        "b2": np.ascontiguousarray(np.asarray(inputs["b2"], np.float32)),
        "ident16": np.eye(P, dtype=np.float16),
        "ones11": np.ones((1, 1), np.float32),
        "ones_row": np.ones((1, P), np.float16),
    }

    in_maps = []
    for c in range(N_CORES):
        hvc = np.zeros((SEGS_PER_CORE, CAP_SEG, D), np.float16)
        ab = np.zeros((P, SEGS_PER_CORE), np.float16)
        for t in range(SEGS_PER_CORE):
            s = c * SEGS_PER_CORE + t
            lo, hi = bounds[s], bounds[s + 1]
            hvc[t, : hi - lo] = h16[lo:hi]
            ab[:, t] = 64.0 / max(hi - lo, 1)
        hv_core = np.ascontiguousarray(
            hvc.reshape(SEGS_PER_CORE, P, ROWS_PP, D).transpose(1, 0, 2, 3)
        ).reshape(P, Q * D)
        in_maps.append({"hv16": hv_core, "abar": ab, **weights})

    nc = _get_nc()
    res = bass_utils.run_bass_kernel_spmd(
        nc,
        in_maps,
        core_ids=list(range(N_CORES)),
        trace=trace,
        **(trace_kwargs or {}),
    )

    out_full = np.empty((n, D), np.float32)
    for c in range(N_CORES):
        o = (
            np.asarray(res.results[c]["out"])
            .astype(np.float32)
            .reshape(P, SEGS_PER_CORE, ROWS_PP, D)
            .transpose(1, 0, 2, 3)
            .reshape(SEGS_PER_CORE, CAP_SEG, D)
        )
        for t in range(SEGS_PER_CORE):
            s = c * SEGS_PER_CORE + t
            lo, hi = bounds[s], bounds[s + 1]
            out_full[lo:hi] = o[t, : hi - lo]
    return out_full, res


def kernel(**inputs) -> np.ndarray:
    out, _ = run(inputs, trace=False)
    return out


# revision 10
# speedup vs baseline: 1.4080x; 1.3108x over previous
"""Trainium2 Bass kernel: segment-mean -> gated MLP -> per-node modulation.

Computes, for h_V [N, D] and sorted batch_id [N] (values in [0, S)):
    seg_sum[s] = sum of h_V rows with batch_id == s ; counts[s]
    c_V = seg_sum / max(counts, 1)
    g   = sigmoid(relu(c_V @ W1 + b1) @ W2 + b2)
    out = h_V * g[batch_id]

Distribution: batch_id is SORTED, so rows of each segment are contiguous.
We shard by WHOLE segments: core c owns segments [8c, 8c+8) -- every
segment's mean is core-local, so there are NO collectives at all.

Per-core layout (host-marshalled, pure layout/dtype transform): each
segment t gets a fixed region of 128 rows x 128 partitions (16384-row
capacity, zero-padded).  Every SBUF partition therefore holds rows of
exactly ONE segment per region, which collapses the segment reduction to
a stream of wide accumulating matmuls with a constant [P,1] weight
column (64/R_t, so PSUM accumulates 64*mean directly; the 1/64 is folded
into W1 on the host).  The gate gather likewise collapses to a rank-1
broadcast matmul per segment, and the modulation pass reuses the SBUF-
resident fp16 tiles from pass 1 (zero re-read of h_V).

Only h_V's fp16 rounding (~5e-4 relative) is lossy; the harness
tolerance is 2e-2.
"""

import math

import numpy as np

# Problem constants (hardcoded per the harness contract).
D = 128  # feature dim
S = 64  # number of segments
P = 128  # SBUF partitions
N_CORES = 8
N_FULL = 1_000_000
SEGS_PER_CORE = S // N_CORES  # 8
ROWS_PP = 128  # rows per partition per segment region
CAP_SEG = P * ROWS_PP  # 16384-row capacity per segment
Q = SEGS_PER_CORE * ROWS_PP  # 1024 rows per partition per core
T_ROWS = 32  # rows per partition per macro DMA tile
MACROS_PER_SEG = ROWS_PP // T_ROWS  # 4
SEG_ELS = ROWS_PP * D  # 16384 fp16 els per partition per segment
MAC_ELS = T_ROWS * D  # 4096
CHUNK = 512  # fp16 els per matmul rhs (one f32 PSUM bank of output)
CH_PER_MAC = MAC_ELS // CHUNK  # 8


def segment_kernel(tc, outs, ins):
    """Emit the per-core Tile program (no cross-core communication)."""
    import concourse.mybir as mybir
    from concourse.bass import broadcast_tensor_aps

    nc = tc.nc
    F32 = mybir.dt.float32
    F16 = mybir.dt.float16
    AF = mybir.ActivationFunctionType
    OP = mybir.AluOpType

    hv = ins["hv16"]  # [P, Q*D] f16; per-partition: seg t, row i, d
    abar = ins["abar"]  # [P, SEGS_PER_CORE] f16: col t = 64/R_t
    w1q = ins["W1q"]  # [D, D] f32 = W1 / 64
    w2 = ins["W2"]  # [D, D] f32
    b1 = ins["b1"]  # [D] f32
    b2 = ins["b2"]  # [D] f32
    ident16 = ins["ident16"]  # [P, P] f16 identity
    ones11 = ins["ones11"]  # [1, 1] f32
    ones_row = ins["ones_row"]  # [1, P] f16
    out = outs["out"]  # [P, Q*D] f16 (host upcasts to f32)

    with tc.tile_pool(name="pers", bufs=1) as pers:
        abar_sb = pers.tile_from(abar, name="abar_sb", force_copy=True)
        w1_sb = pers.tile_from(w1q, name="w1_sb", force_copy=True)
        w2_sb = pers.tile_from(w2, name="w2_sb", force_copy=True)
        ident_sb = pers.tile_from(ident16, name="ident_sb", force_copy=True)
        ones11_sb = pers.tile_from(ones11, name="ones11_sb", force_copy=True)
        onesrow_sb = pers.tile_from(ones_row, name="onesrow_sb", force_copy=True)
        b1_sb = pers.tile([P, 1], F32, name="b1_sb")
        nc.sync.dma_start(out=b1_sb, in_=b1)
        b2_sb = pers.tile([P, 1], F32, name="b2_sb")
        nc.sync.dma_start(out=b2_sb, in_=b2)

        with (
            tc.tile_pool(name="hvp", bufs=3 * MACROS_PER_SEG) as hvp,
            tc.tile_pool(name="outp", bufs=6) as outp,
            tc.tile_pool(name="gatep", bufs=2) as gatep,
            tc.tile_pool(name="mlpsb", bufs=2) as mlpsb,
            tc.tile_pool(name="accps", bufs=3, space="PSUM") as accps,
            tc.tile_pool(name="mlpps", bufs=2, space="PSUM") as mlpps,
            tc.tile_pool(name="gateps", bufs=2, space="PSUM") as gateps,
        ):
            def pass1(t):
                """Stream segment t's macros; accumulate 64*mean_t in PSUM."""
                base = t * SEG_ELS
                acc = accps.tile([1, CHUNK], F32, tag="acc", name=f"acc{t}")
                hv_tiles = []
                n_ch = MACROS_PER_SEG * CH_PER_MAC
                ci = 0
                for m in range(MACROS_PER_SEG):
                    hv_t = hvp.tile([P, MAC_ELS], F16, tag="hv", name=f"hv{t}_{m}")
                    lo = base + m * MAC_ELS
                    nc.sync.dma_start(out=hv_t, in_=hv[:, lo : lo + MAC_ELS])
                    hv_tiles.append(hv_t)
                    for ch in range(CH_PER_MAC):
                        nc.tensor.matmul(
                            acc,
                            lhsT=abar_sb[:, t : t + 1],
                            rhs=hv_t[:, ch * CHUNK : (ch + 1) * CHUNK],
                            start=(ci == 0),
                            stop=(ci == n_ch - 1),
                            skip_group_check=True,
                        )
                        ci += 1
                # fold the CHUNK//D phases: cv_row = 64*mean_t [1, D].
                # Emitted here so it queues on DVE BEFORE the previous
                # segment's multiplies are enqueued — the MLP chain then
                # overlaps those multiplies instead of waiting behind them.
                cv_row = mlpsb.tile([1, D], F32, tag="cv", name=f"cv{t}")
                acc_v = acc.rearrange("p (g d) -> p d g", d=D)
                nc.vector.reduce_sum(
                    out=cv_row, in_=acc_v, axis=mybir.AxisListType.X
                )
                return cv_row, hv_tiles

            def mlp_and_pass2(t, cv_row, hv_tiles):
                base = t * SEG_ELS
                # tiny per-segment MLP
                cvt_ps = mlpps.tile([D, 1], F32, tag="mlp", name=f"cvt_ps{t}")
                nc.tensor.matmul(cvt_ps, lhsT=cv_row, rhs=ones11_sb)
                cvt_sb = mlpsb.tile([D, 1], F32, tag="cvt", name=f"cvt{t}")
                nc.scalar.copy(cvt_sb, cvt_ps)
                h1_ps = mlpps.tile([D, 1], F32, tag="mlp", name=f"h1_ps{t}")
                nc.tensor.matmul(h1_ps, lhsT=w1_sb, rhs=cvt_sb)
                h1_sb = mlpsb.tile([D, 1], F32, tag="h1", name=f"h1{t}")
                nc.scalar.activation(h1_sb, h1_ps, AF.Relu, bias=b1_sb, scale=1.0)
                h2_ps = mlpps.tile([D, 1], F32, tag="mlp", name=f"h2_ps{t}")
                nc.tensor.matmul(h2_ps, lhsT=w2_sb, rhs=h1_sb)
                g_col = mlpsb.tile([D, 1], F16, tag="gc", name=f"gcol{t}")
                nc.scalar.activation(g_col, h2_ps, AF.Sigmoid, bias=b2_sb, scale=1.0)
                # g as a row: [1, D] = g_col^T via identity
                grow_ps = mlpps.tile([1, D], F32, tag="mlp", name=f"grow_ps{t}")
                nc.tensor.matmul(grow_ps, lhsT=g_col, rhs=ident_sb)
                g_row = mlpsb.tile([1, D], F16, tag="gr", name=f"grow{t}")
                nc.scalar.copy(g_row, grow_ps)
                # broadcast to all partitions: gate[p, d] = g[d]
                gate_ps = gateps.tile([P, D], F32, tag="gps", name=f"gate_ps{t}")
                nc.tensor.matmul(gate_ps, lhsT=onesrow_sb, rhs=g_row)
                gate_sb = gatep.tile([P, D], F16, tag="gate", name=f"gate{t}")
                nc.scalar.copy(gate_sb, gate_ps)

                # pass 2: modulate the retained fp16 tiles, store
                gate3 = gate_sb.rearrange("p (o d) -> p o d", o=1)
                for m in range(MACROS_PER_SEG):
                    out_t = outp.tile([P, MAC_ELS], F16, tag="out", name=f"o{t}_{m}")
                    hv3 = hv_tiles[m].rearrange("p (r d) -> p r d", d=D)
                    in1, in2 = broadcast_tensor_aps(hv3, gate3)
                    nc.vector.tensor_tensor(
                        out_t.rearrange("p (r d) -> p r d", d=D), in1, in2, OP.mult
                    )
                    lo = base + m * MAC_ELS
                    nc.sync.dma_start(out=out[:, lo : lo + MAC_ELS], in_=out_t)

            # Software pipeline: segment t's pass 1 streams while segment
            # t-1 runs its MLP + modulation, so the serial MLP chain never
            # blocks the tensor/DMA stream of the next segment.
            pending = None
            for t in range(SEGS_PER_CORE):
                state = pass1(t)
                if pending is not None:
                    mlp_and_pass2(t - 1, *pending)
                pending = state
            mlp_and_pass2(SEGS_PER_CORE - 1, *pending)


def build_nc():
    """Build the Bass module with ExternalInput/Output DRAM tensors."""
    import concourse.bacc as bacc
    import concourse.mybir as mybir
    import concourse.tile as tile

    F32 = mybir.dt.float32
    F16 = mybir.dt.float16
    nc = bacc.Bacc(
        "TRN2",
        target_bir_lowering=False,
        debug=False,
        enable_asserts=False,
        num_devices=N_CORES,
    )

    def din(name, shape, dt):
        return nc.dram_tensor(name, shape, dt, kind="ExternalInput").ap()

    ins = {
        "hv16": din("hv16", [P, Q * D], F16),
        "abar": din("abar", [P, SEGS_PER_CORE], F16),
        "W1q": din("W1q", [D, D], F32),
        "W2": din("W2", [D, D], F32),
        "b1": din("b1", [D], F32),
        "b2": din("b2", [D], F32),
        "ident16": din("ident16", [P, P], F16),
        "ones11": din("ones11", [1, 1], F32),
        "ones_row": din("ones_row", [1, P], F16),
    }
    outs = {"out": nc.dram_tensor("out", [P, Q * D], F16, kind="ExternalOutput").ap()}
    with tile.TileContext(nc) as tc:
        segment_kernel(tc, outs, ins)
    nc.compile()
    return nc


_NC_CACHE = {}


def _get_nc():
    if "nc" not in _NC_CACHE:
        _NC_CACHE["nc"] = build_nc()
    return _NC_CACHE["nc"]


def run(inputs, trace=False, trace_kwargs=None):
    from concourse import bass_utils

    h_V = np.asarray(inputs["h_V"], dtype=np.float32)
    bid = np.asarray(inputs["batch_id"]).astype(np.int64)
    n = h_V.shape[0]
    counts = np.bincount(bid, minlength=S)
    assert counts.max() <= CAP_SEG, f"segment too large: {counts.max()}"
    bounds = np.concatenate([[0], np.cumsum(counts)])
    h16 = h_V.astype(np.float16)

    weights = {
        "W1q": np.ascontiguousarray(np.asarray(inputs["W1"], np.float32)) / 64.0,
        "W2": np.ascontiguousarray(np.asarray(inputs["W2"], np.float32)),
        "b1": np.ascontiguousarray(np.asarray(inputs["b1"], np.float32)),
        "b2": np.ascontiguousarray(np.asarray(inputs["b2"], np.float32)),
        "ident16": np.eye(P, dtype=np.float16),
        "ones11": np.ones((1, 1), np.float32),
        "ones_row": np.ones((1, P), np.float16),
    }

    in_maps = []
    for c in range(N_CORES):
        hvc = np.zeros((SEGS_PER_CORE, CAP_SEG, D), np.float16)
        ab = np.zeros((P, SEGS_PER_CORE), np.float16)
        for t in range(SEGS_PER_CORE):
            s = c * SEGS_PER_CORE + t
            lo, hi = bounds[s], bounds[s + 1]
            hvc[t, : hi - lo] = h16[lo:hi]
            ab[:, t] = 64.0 / max(hi - lo, 1)
        hv_core = np.ascontiguousarray(
            hvc.reshape(SEGS_PER_CORE, P, ROWS_PP, D).transpose(1, 0, 2, 3)
        ).reshape(P, Q * D)
        in_maps.append({"hv16": hv_core, "abar": ab, **weights})

    nc = _get_nc()
    res = bass_utils.run_bass_kernel_spmd(
        nc,
        in_maps,
        core_ids=list(range(N_CORES)),
        trace=trace,
        **(trace_kwargs or {}),
    )

    out_full = np.empty((n, D), np.float32)
    for c in range(N_CORES):
        o = (
            np.asarray(res.results[c]["out"])
            .astype(np.float32)
            .reshape(P, SEGS_PER_CORE, ROWS_PP, D)
            .transpose(1, 0, 2, 3)
            .reshape(SEGS_PER_CORE, CAP_SEG, D)
        )
        for t in range(SEGS_PER_CORE):
            s = c * SEGS_PER_CORE + t
            lo, hi = bounds[s], bounds[s + 1]
            out_full[lo:hi] = o[t, : hi - lo]
    return out_full, res


def kernel(**inputs) -> np.ndarray:
    out, _ = run(inputs, trace=False)
    return out


# revision 11
# speedup vs baseline: 1.7091x; 1.2139x over previous
"""Trainium2 Bass kernel: segment-mean -> gated MLP -> per-node modulation.

Computes, for h_V [N, D] and sorted batch_id [N] (values in [0, S)):
    seg_sum[s] = sum of h_V rows with batch_id == s ; counts[s]
    c_V = seg_sum / max(counts, 1)
    g   = sigmoid(relu(c_V @ W1 + b1) @ W2 + b2)
    out = h_V * g[batch_id]

Distribution: batch_id is SORTED, so rows of each segment are contiguous.
We shard by WHOLE segments (8 per core, size-ranked so same-rank segments
share a slot across cores) -- every segment's mean is core-local, so
there are NO collectives at all.

Per-core layout (host-marshalled, pure layout/dtype transform): slot t
gets a region of caps[t] rows x 128 partitions (caps[t] =
ceil(max-count-in-slot / 128), zero-padded).  Every SBUF partition holds
rows of exactly ONE segment per region, which collapses the segment
reduction to a stream of wide accumulating matmuls with a constant [P,1]
weight column (64/R_t, so PSUM accumulates 64*mean directly; the 1/64 is
folded into W1 on the host).  The gate gather collapses to a rank-1
broadcast matmul per segment, and the modulation pass reuses the SBUF-
resident fp16 tiles from pass 1 (zero re-read of h_V).  Output is
written fp16 and upcast on the host.

fp16 rounding of h_V and of the output (~1e-3 relative combined) is the
only loss; the harness tolerance is 2e-2.
"""

import math

import numpy as np

# Problem constants (hardcoded per the harness contract).
D = 128  # feature dim
S = 64  # number of segments
P = 128  # SBUF partitions
N_CORES = 8
SEGS_PER_CORE = S // N_CORES  # 8
T_ROWS = 32  # max rows per partition per macro DMA tile
MAC_ELS = T_ROWS * D  # 4096
CHUNK = 512  # fp16 els per matmul rhs (one f32 PSUM bank of output)


def _macro_rows(cap):
    """Split cap rows/partition into macro tiles of <=32 rows."""
    rows = [T_ROWS] * (cap // T_ROWS)
    if cap % T_ROWS:
        rows.append(cap % T_ROWS)
    return rows


def _chunks(els, first_macro_of_slot, last_macro_of_slot):
    """Chunk element counts for one macro; remainder chunk ordered so the
    globally-first chunk is full (start flag zeroes the whole bank) and
    the globally-last chunk is full."""
    full, rem = divmod(els, CHUNK)
    out = [CHUNK] * full
    if rem:
        if last_macro_of_slot and full:
            out = [rem] + out  # keep a full chunk last
        else:
            out = out + [rem]
    return out


def segment_kernel(tc, outs, ins, caps):
    """Emit the per-core Tile program (no cross-core communication)."""
    import concourse.mybir as mybir
    from concourse.bass import broadcast_tensor_aps

    nc = tc.nc
    F32 = mybir.dt.float32
    F16 = mybir.dt.float16
    AF = mybir.ActivationFunctionType
    OP = mybir.AluOpType

    hv = ins["hv16"]  # [P, TOT_ELS] f16; per-partition: slot t, row i, d
    abar = ins["abar"]  # [P, SEGS_PER_CORE] f16: col t = 64/R_t
    w1q = ins["W1q"]  # [D, D] f32 = W1 / 64
    w2 = ins["W2"]  # [D, D] f32
    b1 = ins["b1"]  # [D] f32
    b2 = ins["b2"]  # [D] f32
    ident16 = ins["ident16"]  # [P, P] f16 identity
    ones11 = ins["ones11"]  # [1, 1] f32
    ones_row = ins["ones_row"]  # [1, P] f16
    out = outs["out"]  # [P, TOT_ELS] f16 (host upcasts to f32)

    bases = [0]
    for cap in caps:
        bases.append(bases[-1] + cap * D)

    with tc.tile_pool(name="pers", bufs=1) as pers:
        abar_sb = pers.tile_from(abar, name="abar_sb", force_copy=True)
        w1_sb = pers.tile_from(w1q, name="w1_sb", force_copy=True)
        w2_sb = pers.tile_from(w2, name="w2_sb", force_copy=True)
        ident_sb = pers.tile_from(ident16, name="ident_sb", force_copy=True)
        ones11_sb = pers.tile_from(ones11, name="ones11_sb", force_copy=True)
        onesrow_sb = pers.tile_from(ones_row, name="onesrow_sb", force_copy=True)
        b1_sb = pers.tile([P, 1], F32, name="b1_sb")
        nc.sync.dma_start(out=b1_sb, in_=b1)
        b2_sb = pers.tile([P, 1], F32, name="b2_sb")
        nc.sync.dma_start(out=b2_sb, in_=b2)

        with (
            tc.tile_pool(name="hvp", bufs=16) as hvp,
            tc.tile_pool(name="outp", bufs=6) as outp,
            tc.tile_pool(name="gatep", bufs=2) as gatep,
            tc.tile_pool(name="mlpsb", bufs=2) as mlpsb,
            tc.tile_pool(name="accps", bufs=3, space="PSUM") as accps,
            tc.tile_pool(name="mlpps", bufs=2, space="PSUM") as mlpps,
            tc.tile_pool(name="gateps", bufs=2, space="PSUM") as gateps,
        ):

            def pass1(t):
                """Stream slot t's macros; accumulate 64*mean_t in PSUM."""
                base = bases[t]
                macs = _macro_rows(caps[t])
                acc = accps.tile([1, CHUNK], F32, tag="acc", name=f"acc{t}")
                hv_tiles = []
                chunk_lists = [
                    _chunks(r * D, m == 0, m == len(macs) - 1)
                    for m, r in enumerate(macs)
                ]
                n_ch = sum(len(cl) for cl in chunk_lists)
                ci = 0
                lo = base
                for m, r in enumerate(macs):
                    els = r * D
                    hv_t = hvp.tile([P, MAC_ELS], F16, tag="hv", name=f"hv{t}_{m}")
                    nc.sync.dma_start(out=hv_t[:, :els], in_=hv[:, lo : lo + els])
                    hv_tiles.append((hv_t, els, lo))
                    off = 0
                    for ch in chunk_lists[m]:
                        nc.tensor.matmul(
                            acc[:, :ch],
                            lhsT=abar_sb[:, t : t + 1],
                            rhs=hv_t[:, off : off + ch],
                            start=(ci == 0),
                            stop=(ci == n_ch - 1),
                            skip_group_check=True,
                        )
                        off += ch
                        ci += 1
                    lo += els
                # fold the CHUNK//D phases: cv_row = 64*mean_t [1, D].
                # Emitted here so it queues on DVE BEFORE the previous
                # segment's multiplies are enqueued — the MLP chain then
                # overlaps those multiplies instead of waiting behind them.
                cv_row = mlpsb.tile([1, D], F32, tag="cv", name=f"cv{t}")
                acc_v = acc.rearrange("p (g d) -> p d g", d=D)
                nc.vector.reduce_sum(
                    out=cv_row, in_=acc_v, axis=mybir.AxisListType.X
                )
                return cv_row, hv_tiles

            def mlp_and_pass2(t, cv_row, hv_tiles):
                # tiny per-slot MLP
                cvt_ps = mlpps.tile([D, 1], F32, tag="mlp", name=f"cvt_ps{t}")
                nc.tensor.matmul(cvt_ps, lhsT=cv_row, rhs=ones11_sb)
                cvt_sb = mlpsb.tile([D, 1], F32, tag="cvt", name=f"cvt{t}")
                nc.scalar.copy(cvt_sb, cvt_ps)
                h1_ps = mlpps.tile([D, 1], F32, tag="mlp", name=f"h1_ps{t}")
                nc.tensor.matmul(h1_ps, lhsT=w1_sb, rhs=cvt_sb)
                h1_sb = mlpsb.tile([D, 1], F32, tag="h1", name=f"h1{t}")
                nc.scalar.activation(h1_sb, h1_ps, AF.Relu, bias=b1_sb, scale=1.0)
                h2_ps = mlpps.tile([D, 1], F32, tag="mlp", name=f"h2_ps{t}")
                nc.tensor.matmul(h2_ps, lhsT=w2_sb, rhs=h1_sb)
                g_col = mlpsb.tile([D, 1], F16, tag="gc", name=f"gcol{t}")
                nc.scalar.activation(g_col, h2_ps, AF.Sigmoid, bias=b2_sb, scale=1.0)
                # g as a row: [1, D] = g_col^T via identity
                grow_ps = mlpps.tile([1, D], F32, tag="mlp", name=f"grow_ps{t}")
                nc.tensor.matmul(grow_ps, lhsT=g_col, rhs=ident_sb)
                g_row = mlpsb.tile([1, D], F16, tag="gr", name=f"grow{t}")
                nc.scalar.copy(g_row, grow_ps)
                # broadcast to all partitions: gate[p, d] = g[d]
                gate_ps = gateps.tile([P, D], F32, tag="gps", name=f"gate_ps{t}")
                nc.tensor.matmul(gate_ps, lhsT=onesrow_sb, rhs=g_row)
                gate_sb = gatep.tile([P, D], F16, tag="gate", name=f"gate{t}")
                nc.scalar.copy(gate_sb, gate_ps)

                # pass 2: modulate the retained fp16 tiles, store
                gate3 = gate_sb.rearrange("p (o d) -> p o d", o=1)
                for m, (hv_t, els, lo) in enumerate(hv_tiles):
                    out_t = outp.tile([P, MAC_ELS], F16, tag="out", name=f"o{t}_{m}")
                    r = els // D
                    hv3 = hv_t[:, :els].rearrange("p (r d) -> p r d", d=D)
                    in1, in2 = broadcast_tensor_aps(hv3, gate3)
                    nc.vector.tensor_tensor(
                        out_t[:, :els].rearrange("p (r d) -> p r d", d=D),
                        in1,
                        in2,
                        OP.mult,
                    )
                    nc.sync.dma_start(out=out[:, lo : lo + els], in_=out_t[:, :els])

            # Software pipeline: slot t's pass 1 streams while slot t-1
            # runs its MLP + modulation, so the serial MLP chain never
            # blocks the tensor/DMA stream of the next slot.
            pending = None
            for t in range(SEGS_PER_CORE):
                state = pass1(t)
                if pending is not None:
                    mlp_and_pass2(t - 1, *pending)
                pending = state
            mlp_and_pass2(SEGS_PER_CORE - 1, *pending)


def build_nc(caps):
    """Build the Bass module for the given per-slot capacities."""
    import concourse.bacc as bacc
    import concourse.mybir as mybir
    import concourse.tile as tile

    F32 = mybir.dt.float32
    F16 = mybir.dt.float16
    tot = sum(caps) * D
    nc = bacc.Bacc(
        "TRN2",
        target_bir_lowering=False,
        debug=False,
        enable_asserts=False,
        num_devices=N_CORES,
    )

    def din(name, shape, dt):
        return nc.dram_tensor(name, shape, dt, kind="ExternalInput").ap()

    ins = {
        "hv16": din("hv16", [P, tot], F16),
        "abar": din("abar", [P, SEGS_PER_CORE], F16),
        "W1q": din("W1q", [D, D], F32),
        "W2": din("W2", [D, D], F32),
        "b1": din("b1", [D], F32),
        "b2": din("b2", [D], F32),
        "ident16": din("ident16", [P, P], F16),
        "ones11": din("ones11", [1, 1], F32),
        "ones_row": din("ones_row", [1, P], F16),
    }
    outs = {"out": nc.dram_tensor("out", [P, tot], F16, kind="ExternalOutput").ap()}
    with tile.TileContext(nc) as tc:
        segment_kernel(tc, outs, ins, caps)
    nc.compile()
    return nc


_NC_CACHE = {}


def _get_nc(caps):
    if caps not in _NC_CACHE:
        _NC_CACHE[caps] = build_nc(caps)
    return _NC_CACHE[caps]


def run(inputs, trace=False, trace_kwargs=None):
    from concourse import bass_utils

    h_V = np.asarray(inputs["h_V"], dtype=np.float32)
    bid = np.asarray(inputs["batch_id"]).astype(np.int64)
    n = h_V.shape[0]
    counts = np.bincount(bid, minlength=S)
    bounds = np.concatenate([[0], np.cumsum(counts)])
    # size-ranked slot assignment: slot t of core c gets segment
    # order[8t + c]; capacity per slot = max count in the slot.
    order = np.argsort(-counts, kind="stable")
    caps = tuple(
        int(math.ceil(max(counts[order[8 * t + c]] for c in range(N_CORES)) / P))
        for t in range(SEGS_PER_CORE)
    )
    bases = np.concatenate([[0], np.cumsum([cap * D for cap in caps])])
    h16 = h_V.astype(np.float16)

    weights = {
        "W1q": np.ascontiguousarray(np.asarray(inputs["W1"], np.float32)) / 64.0,
        "W2": np.ascontiguousarray(np.asarray(inputs["W2"], np.float32)),
        "b1": np.ascontiguousarray(np.asarray(inputs["b1"], np.float32)),
        "b2": np.ascontiguousarray(np.asarray(inputs["b2"], np.float32)),
        "ident16": np.eye(P, dtype=np.float16),
        "ones11": np.ones((1, 1), np.float32),
        "ones_row": np.ones((1, P), np.float16),
    }

    tot = sum(caps) * D
    in_maps = []
    for c in range(N_CORES):
        hv_core = np.zeros((P, tot), np.float16)
        ab = np.zeros((P, SEGS_PER_CORE), np.float16)
        for t in range(SEGS_PER_CORE):
            seg = order[8 * t + c]
            lo, hi = bounds[seg], bounds[seg + 1]
            r = hi - lo
            cap = caps[t]
            block = np.zeros((P * cap, D), np.float16)
            block[:r] = h16[lo:hi]
            hv_core[:, bases[t] : bases[t + 1]] = block.reshape(P, cap * D)
            ab[:, t] = 64.0 / max(r, 1)
        in_maps.append({"hv16": hv_core, "abar": ab, **weights})

    nc = _get_nc(caps)
    res = bass_utils.run_bass_kernel_spmd(
        nc,
        in_maps,
        core_ids=list(range(N_CORES)),
        trace=trace,
        **(trace_kwargs or {}),
    )

    out_full = np.empty((n, D), np.float32)
    for c in range(N_CORES):
        o = np.asarray(res.results[c]["out"])
        for t in range(SEGS_PER_CORE):
            seg = order[8 * t + c]
            lo, hi = bounds[seg], bounds[seg + 1]
            r = hi - lo
            cap = caps[t]
            block = o[:, bases[t] : bases[t + 1]].reshape(P * cap, D)
            out_full[lo:hi] = block[:r].astype(np.float32)
    return out_full, res


def kernel(**inputs) -> np.ndarray:
    out, _ = run(inputs, trace=False)
    return out
